# revision 1
# baseline (speedup 1.0000x reference)
"""Locally-connected (masked linear) layer for 8 TRN2 NeuronCores.

y = x @ (W * M)^T + b
  x: [4096, 4096] f32, W/M: [4096, 4096] f32, b: [4096] f32.

Strategy (tensor-parallel over out_features):
  - Each core owns a 512-row shard of W/M (and of the output columns).
  - The contraction dim is the minor dim of both x and W in HBM, so the
    host uploads x^T, W^T, M^T (contraction-major) in bf16; the device
    never needs a transpose.
  - Device: masked_w = W^T * M^T (DVE, bf16, exact since M is 0/1),
    then PE matmuls accumulate y^T = masked_w^T.T @ x^T in fp32 PSUM,
    bias is added per-partition on evacuation, y^T shard DMAs out fp32.
  - Host concatenates the 8 y^T shards and transposes back.
"""

import os

import numpy as np
import ml_dtypes

BATCH = 4096
IN_F = 4096
OUT_F = 4096
N_CORES = 8
O_SHARD = OUT_F // N_CORES  # 512
P = 128                     # SBUF partitions
BG = 512                    # batch columns per PSUM accumulation group
XCH = 4                     # k-tiles per x DMA slab

_BF16 = ml_dtypes.bfloat16
_NC = None
LAST_RESULT = None


def _ensure_axon_hooks_stub():
    """bass_utils' axon trace path imports antenv.axon_hooks, which this
    container's antenv stub lacks. Install a minimal registry so the
    import succeeds (hook None => bass_utils skips tracing gracefully)."""
    import sys
    import types

    try:
        import antenv.axon_hooks  # noqa: F401
        return
    except ImportError:
        pass
    import antenv

    mod = types.ModuleType("antenv.axon_hooks")
    mod._HOOK = None

    def set_axon_ntff_profile_hook(h):
        mod._HOOK = h

    def get_axon_ntff_profile_hook():
        return mod._HOOK

    mod.set_axon_ntff_profile_hook = set_axon_ntff_profile_hook
    mod.get_axon_ntff_profile_hook = get_axon_ntff_profile_hook
    antenv.axon_hooks = mod
    sys.modules["antenv.axon_hooks"] = mod


def _install_real_ntff_hook():
    """Wire the ctypes NTFF profiling hook (normally registered by the
    boot middleware) so run_bass_kernel_spmd(trace=True) works."""
    _ensure_axon_hooks_stub()
    import antenv.axon_hooks as ah

    if ah.get_axon_ntff_profile_hook() is None:
        try:
            from trn_agent_boot.trn_boot import _ntff_profile_via_ctypes

            hook = _ntff_profile_via_ctypes("/opt/axon/libaxon_pjrt.so")
            if hook is not None:
                ah.set_axon_ntff_profile_hook(hook)
        except Exception:
            pass
    try:
        import concourse.bass_utils as bu

        bu.upload_artifacts = lambda tmpdir: "local://" + str(tmpdir)
    except Exception:
        pass


def build_nc(batch=BATCH, in_f=IN_F, o_shard=O_SHARD, bg=BG, xch=XCH):
    import concourse.mybir as mybir
    from concourse import bacc
    from concourse.tile import TileContext

    p = P
    kt = in_f // p          # k tiles along contraction
    oc = o_shard // p       # out-feature chunks of 128
    ng = batch // bg        # batch groups
    bf16 = mybir.dt.bfloat16
    f32 = mybir.dt.float32

    nc = bacc.Bacc()
    xT = nc.declare_dram_parameter("xT", [in_f, batch], bf16, isOutput=False)
    wT = nc.declare_dram_parameter("wT", [in_f, o_shard], bf16, isOutput=False)
    mT = nc.declare_dram_parameter("mT", [in_f, o_shard], bf16, isOutput=False)
    bT = nc.declare_dram_parameter("bT", [p, oc], f32, isOutput=False)
    yT = nc.declare_dram_parameter("yT", [o_shard, batch], f32, isOutput=True)

    xv = xT[:].rearrange("(c p) b -> p c b", p=p)   # [128, kt, batch]
    wv = wT[:].rearrange("(c p) o -> p c o", p=p)   # [128, kt, o_shard]
    mv = mT[:].rearrange("(c p) o -> p c o", p=p)

    with TileContext(nc) as tc:
        with tc.tile_pool(name="const", bufs=1) as cpool, \
             tc.tile_pool(name="stage", bufs=4) as spool, \
             tc.tile_pool(name="xin", bufs=6) as xpool, \
             tc.tile_pool(name="acc", bufs=8, space="PSUM") as ppool, \
             tc.tile_pool(name="out", bufs=4) as opool:

            bias_t = cpool.tile([p, oc], f32)
            nc.sync.dma_start(out=bias_t, in_=bT[:])

            # masked weights, resident in SBUF for the whole kernel
            mw = cpool.tile([p, kt, o_shard], bf16)
            for k in range(kt):
                wst = spool.tile([p, o_shard], bf16, tag="w")
                mst = spool.tile([p, o_shard], bf16, tag="m")
                nc.sync.dma_start(out=wst, in_=wv[:, k, :])
                nc.sync.dma_start(out=mst, in_=mv[:, k, :])
                nc.vector.tensor_mul(out=mw[:, k, :], in0=wst, in1=mst)

            for g in range(ng):
                psums = [ppool.tile([p, bg], f32, tag="ps", name=f"ps{g}_{j}")
                         for j in range(oc)]
                xt = None
                for k in range(kt):
                    if k % xch == 0:
                        xt = xpool.tile([p, xch, bg], bf16, tag="x")
                        nc.sync.dma_start(
                            out=xt,
                            in_=xv[:, k:k + xch, g * bg:(g + 1) * bg],
                        )
                    rhs = xt[:, k % xch, :]
                    for j in range(oc):
                        nc.tensor.matmul(
                            psums[j],
                            mw[:, k, j * p:(j + 1) * p],
                            rhs,
                            start=(k == 0),
                            stop=(k == kt - 1),
                        )
                for j in range(oc):
                    ot = opool.tile([p, bg], f32, tag="o")
                    nc.vector.tensor_scalar_add(
                        out=ot, in0=psums[j], scalar1=bias_t[:, j:j + 1]
                    )
                    nc.sync.dma_start(
                        out=yT[j * p:(j + 1) * p, g * bg:(g + 1) * bg],
                        in_=ot,
                    )
    nc.finalize()
    return nc


def _prep_in_maps(x, weight, bias, myFilter):
    oc = O_SHARD // P
    xTb = np.ascontiguousarray(np.asarray(x, np.float32).T).astype(_BF16)
    in_maps = []
    for c in range(N_CORES):
        rows = slice(c * O_SHARD, (c + 1) * O_SHARD)
        wTb = np.ascontiguousarray(
            np.asarray(weight, np.float32)[rows].T).astype(_BF16)
        mTb = np.ascontiguousarray(
            np.asarray(myFilter, np.float32)[rows].T).astype(_BF16)
        bTb = np.ascontiguousarray(
            np.asarray(bias, np.float32)[rows].reshape(oc, P).T)
        in_maps.append({"xT": xTb, "wT": wTb, "mT": mTb, "bT": bTb})
    return in_maps


def kernel(x, weight, bias, myFilter):
    global _NC, LAST_RESULT
    _ensure_axon_hooks_stub()
    from concourse.bass_utils import run_bass_kernel_spmd

    if _NC is None:
        _NC = build_nc()

    in_maps = _prep_in_maps(x, weight, bias, myFilter)

    kwargs = {}
    if os.environ.get("KERNEL_TRACE") == "1":
        _install_real_ntff_hook()
        kwargs["trace"] = True
        tdir = os.environ.get("KERNEL_TRACE_DIR")
        if tdir:
            kwargs["tmpdir"] = tdir

    res = run_bass_kernel_spmd(_NC, in_maps, list(range(N_CORES)), **kwargs)
    LAST_RESULT = res

    yT = np.concatenate(
        [res.results[c]["yT"] for c in range(N_CORES)], axis=0)
    return np.ascontiguousarray(yT.T)



# revision 2
# speedup vs baseline: 1.1030x; 1.1030x over previous
"""Locally-connected (masked linear) layer for 8 TRN2 NeuronCores.

y = x @ (W * M)^T + b
  x: [4096, 4096] f32, W/M: [4096, 4096] f32, b: [4096] f32.

Strategy (tensor-parallel over out_features):
  - Each core owns a 512-row shard of W/M (and of the output columns).
  - The host uploads contraction-major slabs: w/m in 8 slabs of 4
    k-tiles ([128, 4*512] bf16, 4KB contiguous rows) and x in 64
    per-(group, slab) blocks [128, 4*512] bf16.  Large contiguous
    DMAs keep the per-DMA descriptor-generation cost (~650ns on the
    issuing queue) off the critical path; w/m issue on the SP queue,
    x on the Activation queue so the two streams don't serialize.
  - Device: per slab, masked_w = w*m on DVE (bf16, exact since M is
    0/1) into its own persistent SBUF tile, so the PE can start
    accumulating group 0 as soon as slab 0 is ready instead of
    waiting for the full mask multiply.
  - PE: y^T[j] += mw[k]^T.T @ x[k] accumulated over 32 k-tiles in 4
    fp32 PSUM banks per batch group (8 banks -> 2 groups in flight).
  - Evacuation adds bias per-partition and casts to bf16; one DMA
    per group writes the [128, 4*512] result block.
  - Host reassembles the 8 y^T shards, transposes, casts to f32.
"""

import os

import numpy as np
import ml_dtypes

BATCH = 4096
IN_F = 4096
OUT_F = 4096
N_CORES = 8
O_SHARD = OUT_F // N_CORES  # 512
P = 128                     # SBUF partitions
BG = 512                    # batch columns per PSUM accumulation group
SK = 4                      # k-tiles per w/m/x slab

_BF16 = ml_dtypes.bfloat16
_NC = None
LAST_RESULT = None


def _ensure_axon_hooks_stub():
    """bass_utils' axon trace path imports antenv.axon_hooks, which this
    container's antenv stub lacks. Install a minimal registry so the
    import succeeds (hook None => bass_utils skips tracing gracefully)."""
    import sys
    import types

    try:
        import antenv.axon_hooks  # noqa: F401
        return
    except ImportError:
        pass
    import antenv

    mod = types.ModuleType("antenv.axon_hooks")
    mod._HOOK = None

    def set_axon_ntff_profile_hook(h):
        mod._HOOK = h

    def get_axon_ntff_profile_hook():
        return mod._HOOK

    mod.set_axon_ntff_profile_hook = set_axon_ntff_profile_hook
    mod.get_axon_ntff_profile_hook = get_axon_ntff_profile_hook
    antenv.axon_hooks = mod
    sys.modules["antenv.axon_hooks"] = mod


def _install_real_ntff_hook():
    """Wire the ctypes NTFF profiling hook (normally registered by the
    boot middleware) so run_bass_kernel_spmd(trace=True) works."""
    _ensure_axon_hooks_stub()
    import antenv.axon_hooks as ah

    if ah.get_axon_ntff_profile_hook() is None:
        try:
            from trn_agent_boot.trn_boot import _ntff_profile_via_ctypes

            hook = _ntff_profile_via_ctypes("/opt/axon/libaxon_pjrt.so")
            if hook is not None:
                ah.set_axon_ntff_profile_hook(hook)
        except Exception:
            pass
    try:
        import concourse.bass_utils as bu

        bu.upload_artifacts = lambda tmpdir: "local://" + str(tmpdir)
    except Exception:
        pass


def slab_weights(wT, sk=SK):
    """[in_f, o] contraction-major -> [nslab*P, sk*o] slab layout."""
    in_f, o = wT.shape
    kt = in_f // P
    ns = kt // sk
    return np.ascontiguousarray(
        wT.reshape(ns, sk, P, o).transpose(0, 2, 1, 3).reshape(ns * P, sk * o))


def slab_x(xT, bg=BG, sk=SK):
    """[in_f, batch] -> [ng*nslab*P, sk*bg] per-(group, slab) blocks."""
    in_f, batch = xT.shape
    kt = in_f // P
    ns = kt // sk
    ng = batch // bg
    return np.ascontiguousarray(
        xT.reshape(ns, sk, P, ng, bg).transpose(3, 0, 2, 1, 4)
        .reshape(ng * ns * P, sk * bg))


def unslab_y(yS, o_shard, batch, bg=BG):
    """[ng*P, oc*bg] device layout -> [o_shard, batch] y^T shard."""
    ng = batch // bg
    oc = o_shard // P
    return (yS.reshape(ng, P, oc, bg).transpose(2, 1, 0, 3)
            .reshape(o_shard, batch))


def build_nc(batch=BATCH, in_f=IN_F, o_shard=O_SHARD, bg=BG, sk=SK):
    import concourse.mybir as mybir
    from concourse import bacc
    from concourse.tile import TileContext

    p = P
    kt = in_f // p          # k tiles along contraction
    ns = kt // sk           # w/m/x slabs
    oc = o_shard // p       # out-feature chunks of 128
    ng = batch // bg        # batch groups
    bf16 = mybir.dt.bfloat16
    f32 = mybir.dt.float32

    nc = bacc.Bacc()
    xS = nc.declare_dram_parameter("xS", [ng * ns * p, sk * bg], bf16,
                                   isOutput=False)
    wS = nc.declare_dram_parameter("wS", [ns * p, sk * o_shard], bf16,
                                   isOutput=False)
    mS = nc.declare_dram_parameter("mS", [ns * p, sk * o_shard], bf16,
                                   isOutput=False)
    bT = nc.declare_dram_parameter("bT", [p, oc], f32, isOutput=False)
    yS = nc.declare_dram_parameter("yS", [ng * p, oc * bg], bf16,
                                   isOutput=True)

    xv = xS[:].rearrange("(g s p) w -> g s p w", s=ns, p=p)
    wv = wS[:].rearrange("(s p) w -> s p w", p=p)
    mv = mS[:].rearrange("(s p) w -> s p w", p=p)
    yv = yS[:].rearrange("(g p) w -> g p w", p=p)

    with TileContext(nc) as tc:
        with tc.tile_pool(name="const", bufs=1) as cpool, \
             tc.tile_pool(name="stage", bufs=6) as spool, \
             tc.tile_pool(name="xin", bufs=10) as xpool, \
             tc.tile_pool(name="acc", bufs=8, space="PSUM") as ppool, \
             tc.tile_pool(name="out", bufs=3) as opool:

            bias_t = cpool.tile([p, oc], f32)
            nc.sync.dma_start(out=bias_t, in_=bT[:])

            # masked weight slabs, each resident in SBUF for the whole
            # kernel; fine-grained tiles let matmuls start on slab 0
            # while later slabs still stream in.
            mws = []
            for s in range(ns):
                wst = spool.tile([p, sk * o_shard], bf16, tag="w")
                mst = spool.tile([p, sk * o_shard], bf16, tag="m")
                nc.sync.dma_start(out=wst, in_=wv[s])
                nc.sync.dma_start(out=mst, in_=mv[s])
                mw = cpool.tile([p, sk * o_shard], bf16, tag=f"mw{s}")
                nc.vector.tensor_mul(out=mw, in0=wst, in1=mst)
                mws.append(mw)

            xtiles = {}

            def issue_x(g):
                for s in range(ns):
                    t = xpool.tile([p, sk * bg], bf16, tag="x",
                                   name=f"x{g}_{s}")
                    nc.scalar.dma_start(out=t, in_=xv[g, s])
                    xtiles[(g, s)] = t

            issue_x(0)
            for g in range(ng):
                if g + 1 < ng:
                    issue_x(g + 1)
                psums = [ppool.tile([p, bg], f32, tag="ps",
                                    name=f"ps{g}_{j}")
                         for j in range(oc)]
                for s in range(ns):
                    xt = xtiles.pop((g, s))
                    for kk in range(sk):
                        rhs = xt[:, kk * bg:(kk + 1) * bg]
                        for j in range(oc):
                            nc.tensor.matmul(
                                psums[j],
                                mws[s][:, kk * o_shard + j * p:
                                       kk * o_shard + (j + 1) * p],
                                rhs,
                                start=(s == 0 and kk == 0),
                                stop=(s == ns - 1 and kk == sk - 1),
                            )
                ot = opool.tile([p, oc * bg], bf16, tag="o")
                for j in range(oc):
                    nc.vector.tensor_scalar_add(
                        out=ot[:, j * bg:(j + 1) * bg], in0=psums[j],
                        scalar1=bias_t[:, j:j + 1])
                nc.sync.dma_start(out=yv[g], in_=ot)
    nc.finalize()
    return nc


def _prep_in_maps(x, weight, bias, myFilter):
    oc = O_SHARD // P
    xb = np.asarray(x, np.float32).astype(_BF16)
    xSb = slab_x(xb.T)
    in_maps = []
    for c in range(N_CORES):
        rows = slice(c * O_SHARD, (c + 1) * O_SHARD)
        wSb = slab_weights(
            np.asarray(weight, np.float32)[rows].T.astype(_BF16))
        mSb = slab_weights(
            np.asarray(myFilter, np.float32)[rows].T.astype(_BF16))
        bTb = np.ascontiguousarray(
            np.asarray(bias, np.float32)[rows].reshape(oc, P).T)
        in_maps.append({"xS": xSb, "wS": wSb, "mS": mSb, "bT": bTb})
    return in_maps


def kernel(x, weight, bias, myFilter):
    global _NC, LAST_RESULT
    _ensure_axon_hooks_stub()
    from concourse.bass_utils import run_bass_kernel_spmd

    if _NC is None:
        _NC = build_nc()

    in_maps = _prep_in_maps(x, weight, bias, myFilter)

    kwargs = {}
    if os.environ.get("KERNEL_TRACE") == "1":
        _install_real_ntff_hook()
        kwargs["trace"] = True
        tdir = os.environ.get("KERNEL_TRACE_DIR")
        if tdir:
            kwargs["tmpdir"] = tdir

    res = run_bass_kernel_spmd(_NC, in_maps, list(range(N_CORES)), **kwargs)
    LAST_RESULT = res

    yT = np.concatenate(
        [unslab_y(np.asarray(res.results[c]["yS"]), O_SHARD, BATCH)
         for c in range(N_CORES)], axis=0)
    return yT.T.astype(np.float32)
